# revision 74
# baseline (speedup 1.0000x reference)
"""Multi-head attention (B=4, S=2048, D=1024, H=16) on 8 Trainium2 cores.

Sharding: core c computes batch b = c // 2, head group hg = c % 2 (8 heads).
Each core runs the full pipeline for its (b, hg); the host sums the two
partial out-projection results per batch (linear in the head dim).

Kernel structure (per core):
  * QKV projections in fp8 (e4m3) DoubleRow matmuls: host pre-splits x (*4)
    and W (*128) into hi+lo fp8 pairs; each DoubleRow instruction carries two
    rank-128 contraction slots, and the hi*hi / lo*hi / hi*lo products (lo*lo
    dropped, ~2.6e-4 relative) pack 3 slots per 2 contraction chunks ->
    0.75x the fp16 PE cost. PSUM = 512*(x@W); evictions scale by 1/512.
  * Scores stay fp16 (single rank-64 chunk per output block: DoubleRow slot
    packing cannot help), computed as S^T [k, q] per head pair, exp on ACT
    -> E^T tiles [128k, 2h*512q] fp16 (with a constant ln(1/ALPHA) bias so
    the optional DVE exp path matches; the softmax denominator cancels it).
  * Context via stationary-E orientation: out ctx [128 q, 65] = E-chunk^T
    [k,128q]-stationary x [V | ones] moving -> PE cost 65 cycles per
    rank-128 key block (vs 512 in the V-stationary orientation). PSUM col 64
    accumulates the softmax denominator; normalization is a [128,1]
    reciprocal + per-partition scalar multiply, written as [q, 2h*64] f16
    and DMA-transposed (xbar) into the ctxT [d, q] layout the out-projection
    needs as its stationary operand.
  * Out-projection fp16.
  * Emission scheduling: PE is strictly in-order and the exp pipeline is
    paced by 2 score-PSUM buffers, so all non-score PE work (projections,
    ctx groups, out-projection) is split into ~0.1-0.22us micro-units and
    drained between score matmuls by a budget scheduler (~0.6us/key-block,
    deficit carryover).  Phases walk pr-major so projection deadlines
    spread evenly; a release-step gate paces projection DMA; ctx units gate
    on kb>=2 so they never wait on the previous phase's last exp.  A
    deadline-aware hurry-up overdrains the last kbs of a phase when the
    next phase's projection markers are still pending, shrinking the
    boundary force rips that starve ACT.
  * One key block per phase (kb 10) runs its exp on the DVE instead of ACT
    (int32 Schraudolph bit-trick + quadratic mantissa correction, ~0.2%
    max err): op1 converts the PSUM scores inline; the 4 tail ops are
    spread over kbs 9-15 via the work queue so no PSUM-releasing eviction
    is delayed by more than one op, and flushed unconditionally at phase
    end (emission-order RAW: the et write must be emitted before the next
    phase's ctx groups read it).  The ctx accumulation order puts the
    offloaded kb last so its E tile has the whole previous phase to land.
    This takes ACT (the critical engine, 256 exps ~1.04us each) off the
    critical path for one kb per phase.
"""

import numpy as np

B, S, D, H = 4, 2048, 1024, 16
HD = D // H          # 64
NHL = 8              # heads per core
DHG = NHL * HD       # 512 head-group width
HDA = HD + 1         # augmented head dim (ones column)
P = 128
N_CORES = 8

SX = 4.0             # host fp8 pre-scale for activations
SW = 128.0           # host fp8 pre-scale for weights
SEV = 1.0 / (SX * SW)  # eviction scale restoring natural units

# DVE-path exp (Schraudolph bit-trick with quadratic mantissa correction):
# z = int32(2^23*(log2e*s/8 + 127)); y = 2^i*(m^2 + BETA*m + GAMMA) where
# m = 1+frac in [1,2) via mantissa mask/or; y ~= (1/ALPHA)*e^(s/8) with
# 0.2% max error.  The ACT path applies bias ln(1/ALPHA) so both engines
# produce identically scaled E; the softmax denominator cancels the scale.
EXP_BETA = -0.049435831132835006
EXP_GAMMA = 2.020485350629873
EXP_ALPHA = 0.3371619879706471
EXP_A = float(2 ** 23 * np.log2(np.e) / 8.0)
EXP_B = float(2 ** 23 * 127)
EXP_BIAS = float(-np.log(EXP_ALPHA))

_CACHE = {}


_DEFAULT_OPTS = {
    "dve_kbs": (10,),       # kbs per phase whose exp runs on DVE
    "pool_kbs": (),         # kbs per phase whose exp tail runs on Pool
    "op_copy_engine": "vector",   # outproj eviction engine
    "v_evict_engine": "vector",   # V projection eviction engine
    "kq_evict_engine": "vector",  # Q/K projection eviction engine
    "csb_mul_engine": "vector",   # ctx normalization multiply engine
    "drain_budget": 600.0,
    "budget_cap": 1200.0,
}


def _build_nc(debug=False, opts=None):
    import concourse.bacc as bacc
    import concourse.mybir as mybir
    from concourse.tile import TileContext

    o = dict(_DEFAULT_OPTS)
    if opts:
        o.update(opts)

    f8 = mybir.dt.float8e4
    f16 = mybir.dt.float16
    f32 = mybir.dt.float32
    i32 = mybir.dt.int32
    EXP = mybir.ActivationFunctionType.Exp
    DR = mybir.MatmulPerfMode.DoubleRow
    ALU = mybir.AluOpType

    nc = bacc.Bacc("TRN2", target_bir_lowering=False, debug=False,
                   num_devices=N_CORES)

    DT = D // P          # 8 contraction chunks for projections
    PT = DHG // P        # 4 head-pair tiles
    QC = S // 512        # 4 query chunks
    KB = S // P          # 16 key blocks
    QS = 4               # 128-query sub-chunks per qc

    # fp8 hi/lo packed inputs: [...,2,...] pair dim innermost-but-one.
    xq8 = nc.dram_tensor("xq8", [D, 2, S], f8, kind="ExternalInput")
    xk8 = nc.dram_tensor("xk8", [D, 2, S], f8, kind="ExternalInput")
    xv8 = nc.dram_tensor("xv8", [D, 2, S], f8, kind="ExternalInput")
    # wq8/wk8 are host-packed partition-major per pt slice:
    # [2(hi/lo), PT, P, DT*128], so a single pt slice is one contiguous
    # [P, 1024B] DMA (no sub-512B descriptor penalty).
    wq8 = nc.dram_tensor("wq8", [2, PT, P, DT * P], f8, kind="ExternalInput")
    wk8 = nc.dram_tensor("wk8", [2, PT, P, DT * P], f8, kind="ExternalInput")
    wv8 = nc.dram_tensor("wv8", [D, 2, DHG], f8, kind="ExternalInput")
    woT = nc.dram_tensor("woT", [DHG, D], f16, kind="ExternalInput")
    out = nc.dram_tensor("out", [S, D], f32, kind="ExternalOutput")

    def _eng(name):
        return {"vector": nc.vector, "gpsimd": nc.gpsimd}[name]

    with TileContext(nc) as tc:
        with (
            tc.tile_pool(name="weights", bufs=1) as wpool,
            tc.tile_pool(name="persist", bufs=1) as persist,
            tc.tile_pool(name="xstream", bufs=o.get("xbufs", 5)) as xpool,
            tc.tile_pool(name="etile", bufs=32) as etpool,
            tc.tile_pool(name="evict", bufs=o.get("ebufs", 5)) as epool,
            tc.tile_pool(name="norm", bufs=10) as npool,
            tc.tile_pool(name="ctxsb", bufs=o.get("csbufs", 4)) as cspool,
            tc.tile_pool(name="expsc", bufs=1) as xppool,
            tc.tile_pool(name="proj_psum", bufs=2, space="PSUM") as proj_psum,
            tc.tile_pool(name="sc_psum", bufs=2, space="PSUM") as sc_psum,
            tc.tile_pool(name="ctx_psum", bufs=2, space="PSUM") as ctx_psum,
        ):
            # -- weights (loads deferred/interleaved by the scheduler) ----
            # wq/wk tiles are pt-major [P, PT, DT, 128] so one pt slice
            # (the startup-critical 256 KB) is a single contiguous DMA.
            wq = [wpool.tile([P, PT, DT, P], f8, name=f"wq{i}")
                  for i in range(2)]
            wk = [wpool.tile([P, PT, DT, P], f8, name=f"wk{i}")
                  for i in range(2)]
            wv = [wpool.tile([P, DT, DHG], f8, name=f"wv{i}") for i in range(2)]
            wo = wpool.tile([P, PT, D], f16)

            def load_w(wt, wd):
                for i in range(2):
                    nc.sync.dma_start(
                        wt[i][:],
                        wd[:, i, :].rearrange("(dt p) m -> p dt m", p=P))

            def load_wo():
                nc.sync.dma_start(
                    wo[:], woT[:].rearrange("(pt p) m -> p pt m", p=P))

            bias_t = persist.tile([P, 1], f32)
            nc.any.memset(bias_t[:], EXP_BIAS)
            # Dummy activation so the exp table load (1.28us) happens at
            # t~0 on the idle ACT engine instead of before the first real
            # exp on the critical path.
            if o.get("warm_act", True):
                warm_t = persist.tile([P, 1], f32)
                nc.scalar.activation(warm_t[:], bias_t[:], EXP, scale=1.0)
            kT = persist.tile([P, PT, S], f16)
            qT = persist.tile([P, PT, S], f16)
            vaug = persist.tile([P, KB, NHL * HDA], f16)
            ctxT = persist.tile([P, PT, S], f16)
            for kb in range(KB):
                nc.any.memset(
                    vaug[:, kb].rearrange("p (h x) -> p h x", h=NHL)[:, :, HD:HDA],
                    1.0)

            def load_x(xdram, qc):
                """x chunk hi/lo pair: two [128, DT, 512] fp8 tiles."""
                xts = []
                for i in range(2):
                    xt = xpool.tile([P, DT, 512], f8, tag="x", name=f"x{i}")
                    nc.sync.dma_start(
                        xt[:],
                        xdram[:, i, qc * 512:(qc + 1) * 512]
                        .rearrange("(dt p) n -> p dt n", p=P))
                    xts.append(xt)
                return xts

            # The 12 DoubleRow instructions of one projection tile, split
            # into micro-slices for the scheduler.  Product-kind-major
            # (all hi*hi chunks first, then lo*hi, then hi*lo) so part 0
            # only needs the hi halves of both operands - at startup the
            # first tiles begin before the lo DMA lands.
            if o.get("mm3_kind_major", False):
                _MM3 = [(c, wi, xi) for (wi, xi) in ((0, 0), (1, 0), (0, 1))
                        for c in range(0, DT, 2)]
            else:
                _MM3 = [(c, wi, xi) for c in range(0, DT, 2)
                        for (wi, xi) in ((0, 0), (1, 0), (0, 1))]

            def mm3_part(ps, w8, x8, pt, part, ipp=4):
                """Stationary = w8 pt slice, moving = x8 (Q/K proj)."""
                for i in range(ipp * part, ipp * part + ipp):
                    c, wi, xi = _MM3[i]
                    nc.tensor.matmul(
                        ps[:], w8[wi][:, pt, c:c + 2, :],
                        x8[xi][:, c:c + 2, :],
                        start=(i == 0), stop=(i == len(_MM3) - 1),
                        perf_mode=DR)

            def mm3v_part(ps, x8, ks, hp, part):
                """Stationary = x8 key-block slice, moving = wv (V proj)."""
                for i in range(4 * part, 4 * part + 4):
                    c, xi, wi = _MM3[i]
                    nc.tensor.matmul(
                        ps[:, 0:P], x8[xi][:, c:c + 2, ks * P:(ks + 1) * P],
                        wv[wi][:, c:c + 2, hp * P:(hp + 1) * P],
                        start=(i == 0), stop=(i == len(_MM3) - 1),
                        perf_mode=DR)

            # -- attention phase pieces -----------------------------------
            _csb_stash = {}

            # ctx accumulation order: offloaded-exp kbs last, so their E
            # tiles (computed by spread-out DVE tail ops) have the whole
            # previous phase to land before any ctx group needs them.
            _off = tuple(o["dve_kbs"]) + tuple(o["pool_kbs"])
            CTX_ORDER = ([k for k in range(KB) if k not in _off] +
                         [k for k in range(KB) if k in _off])
            _H0KC = max(kb // 4 for kb in CTX_ORDER[:KB // 2])

            def ctx_half(pr, qc, ets, h, qs, half, hold):
                """Half of one ctx accumulation group [128q, 65]; the
                second half also does the normalization + transpose."""
                hg = 2 * pr + h
                kbs = CTX_ORDER[:KB // 2] if half == 0 else CTX_ORDER[KB // 2:]
                if half == 0:
                    hold['cps'] = ctx_psum.tile([P, 512], f32, tag="ctx",
                                                name="cps")
                cps = hold['cps']
                for j, kb in enumerate(kbs):
                    gj = j + half * (KB // 2)
                    nc.tensor.matmul(
                        cps[:, 0:HDA],
                        ets[kb][:, h * 512 + qs * P:h * 512 + (qs + 1) * P],
                        vaug[:, kb, hg * HDA:(hg + 1) * HDA],
                        start=(gj == 0), stop=(gj == KB - 1))
                if half == 0:
                    return
                rec = npool.tile([P, 1], f32, tag="rec")
                nc.vector.reciprocal_approx_fast(rec[:], cps[:, HD:HDA])
                if h == 0:
                    csb = cspool.tile([P, P], f16, tag="csb",
                                      name=f"csb_{pr}_{qc}_{qs}")
                    _csb_stash[(pr, qc, qs)] = csb
                else:
                    csb = _csb_stash.pop((pr, qc, qs))
                _eng(o["csb_mul_engine"]).tensor_scalar_mul(
                    csb[:, h * HD:(h + 1) * HD], cps[:, 0:HD], rec[:])
                if h == 1:
                    nc.sync.dma_start_transpose(
                        ctxT[:, pr, qc * 512 + qs * P:qc * 512 + (qs + 1) * P],
                        csb[:])

            def push_ctx(pr, qc, ets, h, qs, min_kb, vrow=None):
                # half 0 reads only the first 8 ets of the previous phase
                # (ready before this phase starts), so it may drain at kb 0
                # where the scheduler otherwise has no eligible work.
                hold = {}
                for half in range(2):
                    pre = None
                    if vrow is not None and (h, qs) == (0, 0):
                        pre = f"V{vrow}c{_H0KC if half == 0 else 3}"
                    work_q.append(
                        (217, (lambda a, b, c, d, e, hf:
                               lambda: ctx_half(a, b, c, d, e, hf, hold))
                         (pr, qc, ets, h, qs, half), None,
                         int(o.get("h0_kb", 2)) if half == 0 else min_kb,
                         pre))

            def push_op_micro(sc_, jc, min_kb, ceng=None):
                hold = {}
                def f_dt(dt):
                    def f():
                        if dt == 0:
                            hold['ps'] = proj_psum.tile(
                                [P, 512], f32, tag="proj", name="ps_op")
                        nc.tensor.matmul(
                            hold['ps'][:], ctxT[:, dt, sc_ * P:(sc_ + 1) * P],
                            wo[:, dt, jc * 512:(jc + 1) * 512],
                            start=(dt == 0), stop=(dt == PT - 1))
                        if dt == PT - 1:
                            ot = epool.tile([P, 512], f32, tag="o")
                            _eng(ceng or o["op_copy_engine"]).tensor_copy(
                                ot[:], hold['ps'][:])
                            nc.sync.dma_start(
                                out[sc_ * P:(sc_ + 1) * P,
                                    jc * 512:(jc + 1) * 512],
                                ot[:])
                    return f
                for dt in range(PT):
                    work_q.append((214, f_dt(dt), None, min_kb))

            def outproj_half(sc_, jc):
                ps = proj_psum.tile([P, 512], f32, tag="proj")
                for dt in range(PT):
                    nc.tensor.matmul(
                        ps[:], ctxT[:, dt, sc_ * P:(sc_ + 1) * P],
                        wo[:, dt, jc * 512:(jc + 1) * 512],
                        start=(dt == 0), stop=(dt == PT - 1))
                ot = epool.tile([P, 512], f32, tag="o")
                _eng(o["op_copy_engine"]).tensor_copy(ot[:], ps[:])
                nc.sync.dma_start(
                    out[sc_ * P:(sc_ + 1) * P, jc * 512:(jc + 1) * 512],
                    ot[:])

            # -- scheduler ------------------------------------------------
            # PE is strictly in-order and sc_psum is only 2 deep, so the
            # score loop runs at ACT's exp pace (~1.04us/kb) leaving
            # ~0.6us/kb of PE slack.  All other PE work (projections, ctx
            # groups, out-projection) is queued as small units and drained
            # between kb steps so neither engine ever waits on the other.
            # work_q (ctx/outproj, FIFO) drains before script_q
            # (projections, ordered with markers for dependency forcing).
            work_q = []      # entries: (ns, fn, marker, min_kb)
            script_q = []    # entries: (ns, fn, marker)
            done_markers = set()
            _bud = [0.0]

            cur_step = [99]

            def drain(add, kb=99):
                # Budget accumulator with deficit carryover: a 1280ns unit
                # drained against a 600ns slot leaves a debt the next slots
                # repay, keeping PE on ACT's pace on average.  work_q units
                # gated on min_kb (ctx groups touch the previous phase's
                # last exp output, which ACT only finishes ~1 kb into this
                # phase - draining them earlier stalls PE on ACT and
                # starves the exp pipeline).
                _bud[0] += add
                while _bud[0] > 0:
                    wi = next((i for i, w in enumerate(work_q)
                               if kb >= w[3]), None)
                    if wi is not None:
                        ent = work_q.pop(wi)
                        ns, fn, mk = ent[:3]
                        if len(ent) > 4 and ent[4]:
                            force(ent[4])
                    elif script_q and script_q[0][3] <= cur_step[0]:
                        ns, fn, mk = script_q.pop(0)[:3]
                    else:
                        break
                    fn()
                    if mk:
                        done_markers.add(mk)
                    _bud[0] -= ns
                if _bud[0] > o["budget_cap"]:
                    _bud[0] = o["budget_cap"]

            def force(mk):
                while mk not in done_markers:
                    ns, fn, m2, _rel = script_q.pop(0)
                    fn()
                    if m2:
                        done_markers.add(m2)

            def exp_dve_ops(zt, et, eng):
                """The 4 Schraudolph tail ops as separate thunks so the
                scheduler can spread them between PSUM-releasing
                evictions on the same (in-order) engine queue."""
                m2 = xppool.tile([P, 1024], i32, tag="xm2", name="m2",
                                 bufs=1)
                a1 = xppool.tile([P, 1024], i32, tag="xa1", name="a1",
                                 bufs=1)
                t = xppool.tile([P, 1024], f32, tag="xt", name="t",
                                bufs=1)
                return [
                    lambda: eng.tensor_scalar(
                        m2[:], zt[:], 0x007FFFFF, 0x3F800000,
                        ALU.bitwise_and, ALU.bitwise_or),
                    lambda: eng.tensor_scalar(
                        a1[:], zt[:], -8388608, 0,
                        ALU.bitwise_and, ALU.bitwise_or),
                    lambda: eng.scalar_tensor_tensor(
                        t[:], m2[:].bitcast(f32), EXP_BETA,
                        m2[:].bitcast(f32), ALU.add, ALU.mult),
                    lambda: eng.scalar_tensor_tensor(
                        et[:], t[:], EXP_GAMMA,
                        a1[:].bitcast(f32), ALU.add, ALU.mult),
                ]

            def phase_scores(pr, qc, dve_kbs=(), pool_kbs=(), next_mks=()):
                ets = []
                hurry_kb = int(o.get("hurry_kb", 10))
                hurry_ns = float(o.get("hurry_ns", 500.0))
                for kb in range(KB):
                    sc = sc_psum.tile([P, 1024], f32, tag="sc")
                    et = etpool.tile([P, 1024], f16, tag="e",
                                     name=f"et_{pr}_{qc}_{kb}")
                    for h in range(2):
                        nc.tensor.matmul(
                            sc[:, h * 512:(h + 1) * 512],
                            kT[h * 64:(h + 1) * 64, pr, kb * P:(kb + 1) * P],
                            qT[h * 64:(h + 1) * 64, pr, qc * 512:(qc + 1) * 512],
                            start=True, stop=True)
                    if kb in dve_kbs or kb in pool_kbs:
                        # op1 reads PSUM: always DVE (gpsimd PSUM access is
                        # unverified); tail runs on the chosen engine.
                        teng = _eng("gpsimd" if kb in pool_kbs else "vector")
                        zt = xppool.tile([P, 1024], i32, tag="xz", name="zt",
                                         bufs=1)
                        nc.vector.tensor_scalar(zt[:], sc[:], EXP_A, EXP_B,
                                                ALU.mult, ALU.add)
                        tkbs = o.get("tail_kbs", (9, 11, 13, 15))
                        for op, tkb in zip(exp_dve_ops(zt, et, teng), tkbs):
                            work_q.append(
                                (0, op, None, max(tkb, min(kb + 2, KB - 1))))
                    else:
                        nc.scalar.activation(et[:], sc[:], EXP,
                                             scale=1.0 / 8.0, bias=bias_t[:])
                    ets.append(et)
                    add = o["drain_budget"]
                    if any(kb - k in (1, 2) for k in dve_kbs):
                        # ACT has no exp for the offloaded kb and op1 holds
                        # the PSUM buffer ~1.2us: fill PE with extra work.
                        add += o.get("hole_extra", 0.0)
                    if (kb >= hurry_kb
                            and any(m not in done_markers for m in next_mks)):
                        # next phase's projection deps are behind schedule:
                        # overdrain now so the boundary force rips less.
                        add += hurry_ns
                    drain(add, kb=kb)
                # Flush zero-cost (DVE exp tail) entries unconditionally:
                # their writes MUST be emitted before the next phase's ctx
                # groups read the offloaded et tile - emission-order RAW.
                i = 0
                while i < len(work_q):
                    if work_q[i][0] == 0:
                        ent = work_q.pop(i)
                        ent[1]()
                        if ent[2]:
                            done_markers.add(ent[2])
                    else:
                        i += 1
                return ets

            # -- projection script ----------------------------------------
            # Micro-units (<=~430ns of PE each) keep filler jitter small
            # relative to the ~610ns/kb slack ACT leaves, so exp never
            # waits long for its next scores tile.  Loads sit ahead of
            # their consumers; x chunks for K rows pr>0 and the per-head-
            # pair V passes are reloaded (extra DMA, big SBUF saving).
            xslot = {}
            sq = script_q
            _rel = [0]
            _rshift = int(o.get("rel_shift", 0))

            def set_rel(r):
                _rel[0] = max(0, r - _rshift)

            def sq_load(key, xdram, qc):
                def f():
                    xslot[key] = load_x(xdram, qc)
                sq.append((0, f, None, _rel[0]))

            def sq_misc(f):
                sq.append((0, f, None, _rel[0]))

            KQ_PARTS = int(o.get("kq_parts", 6))

            def sq_kq(w8, key, dst, pt, qc, mk=None):
                hold = {}
                ipp = 12 // KQ_PARTS
                for part in range(KQ_PARTS):
                    def f(part=part):
                        if part == 0:
                            hold['ps'] = proj_psum.tile(
                                [P, 512], f32, tag="proj", name="ps_u")
                        mm3_part(hold['ps'], w8, xslot[key],
                                 pt, part, ipp=ipp)
                        if part == KQ_PARTS - 1:
                            _eng(o["kq_evict_engine"]).tensor_scalar_mul(
                                dst[:, pt, qc * 512:(qc + 1) * 512],
                                hold['ps'][:], SEV)
                    sq.append((1281 // KQ_PARTS, f,
                               mk if part == KQ_PARTS - 1 else None, _rel[0]))

            def sq_v(key, kc, ks, hp, mk=None):
                hold = {}
                for part in range(3):
                    def f(part=part):
                        if part == 0:
                            hold['ps'] = proj_psum.tile(
                                [P, 512], f32, tag="proj", name="ps_u")
                        mm3v_part(hold['ps'], xslot[key], ks, hp, part)
                        if part == 2:
                            kb = kc * 4 + ks
                            va = vaug[:, kb].rearrange("p (h x) -> p h x",
                                                       h=NHL)
                            _eng(o["v_evict_engine"]).tensor_scalar_mul(
                                va[:, 2 * hp:2 * hp + 2, 0:HD],
                                hold['ps'][:, 0:P]
                                .rearrange("p (h x) -> p h x", h=2), SEV)
                    sq.append((107, f, mk if part == 2 else None, _rel[0]))

            # startup: only what the first score phase needs, DMA-ordered
            # so Q's operands land first.  The wq/wk pt0 slices (256 KB of
            # 1 MB) load before their x chunks so the first Q00/K00 tiles
            # start ~4 us earlier; the pt1-3 slices follow after xk0.
            def load_w_slice(wt, wd, pt):
                for i in range(2):
                    nc.sync.dma_start(
                        wt[i][:, pt],
                        wd[i, pt].rearrange("p (dt m) -> p dt m", m=P))

            def load_w_rest(wt, wd):
                """pt slices 1..PT-1 in one DMA per hi/lo half."""
                for i in range(2):
                    nc.sync.dma_start(
                        wt[i][:, 1:PT].rearrange("p pt dt m -> p pt (dt m)"),
                        wd[i, 1:PT].rearrange("pt p dm -> p pt dm"))

            def load_w_full(wt, wd):
                """whole weight in one DMA per hi/lo half."""
                for i in range(2):
                    nc.sync.dma_start(
                        wt[i][:].rearrange("p pt dt m -> p pt (dt m)"),
                        wd[i, :].rearrange("pt p dm -> p pt dm"))

            wsplit = o.get("wsplit", "full")
            if wsplit == "kfirst":
                load_w_slice(wk, wk8, 0)
                xslot["k0"] = load_x(xk8, 0)
                load_w_slice(wq, wq8, 0)
                xslot["q0"] = load_x(xq8, 0)
                load_w_rest(wq, wq8)
                load_w_rest(wk, wk8)
            elif wsplit == "split":
                load_w_slice(wq, wq8, 0)
                xslot["q0"] = load_x(xq8, 0)
                load_w_slice(wk, wk8, 0)
                xslot["k0"] = load_x(xk8, 0)
                load_w_rest(wq, wq8)
                load_w_rest(wk, wk8)
            else:
                load_w_full(wq, wq8)
                xslot["q0"] = load_x(xq8, 0)
                load_w_full(wk, wk8)
                xslot["k0"] = load_x(xk8, 0)
            if wsplit == "kfirst":
                ps1 = proj_psum.tile([P, 512], f32, tag="proj",
                                     name="ps_k00")
                for part in range(3):
                    mm3_part(ps1, wk, xslot["k0"], 0, part)
                nc.vector.tensor_scalar_mul(kT[:, 0, 0:512], ps1[:], SEV)
                ps0 = proj_psum.tile([P, 512], f32, tag="proj",
                                     name="ps_q00")
                for part in range(3):
                    mm3_part(ps0, wq, xslot["q0"], 0, part)
                nc.vector.tensor_scalar_mul(qT[:, 0, 0:512], ps0[:], SEV)
            else:
                ps0 = proj_psum.tile([P, 512], f32, tag="proj",
                                     name="ps_q00")
                for part in range(3):
                    mm3_part(ps0, wq, xslot["q0"], 0, part)
                nc.vector.tensor_scalar_mul(qT[:, 0, 0:512], ps0[:], SEV)
                ps1 = proj_psum.tile([P, 512], f32, tag="proj",
                                     name="ps_k00")
                for part in range(3):
                    mm3_part(ps1, wk, xslot["k0"], 0, part)
                nc.vector.tensor_scalar_mul(kT[:, 0, 0:512], ps1[:], SEV)

            # script, in drain order aligned with the pr-major phase walk.
            # Deadlines: Q(pr,qc) at phase (pr,qc); K{pr} and Q(pr,0) at
            # row start (pr,0); V{pr} when C(pr,0) drains one phase later.
            # Row pr's K/V x chunks are reloaded per pass (DMA for SBUF).
            LOOKAHEAD = int(o.get("lookahead", 1))

            def k_pass(pt, mk, skip0=False):
                """K row pt with LOOKAHEAD-chunk load lookahead."""
                kcs = list(range(1 if skip0 else 0, QC))
                keys = [f"k{pt}_{kc}" for kc in kcs]
                for j in range(min(LOOKAHEAD, len(kcs))):
                    sq_load(keys[j], xk8, kcs[j])
                for j, kc in enumerate(kcs):
                    if j + LOOKAHEAD < len(kcs):
                        sq_load(keys[j + LOOKAHEAD], xk8, kcs[j + LOOKAHEAD])
                    sq_kq(wk, keys[j], kT, pt, kc,
                          mk=mk if j == len(kcs) - 1 else None)

            def v_pass(hp, mk=None):
                for kc in range(min(LOOKAHEAD, QC)):
                    sq_load(f"v{hp}_{kc}", xv8, kc)
                for kc in range(QC):
                    if kc + LOOKAHEAD < QC:
                        sq_load(f"v{hp}_{kc + LOOKAHEAD}", xv8,
                                kc + LOOKAHEAD)
                    for ks in range(4):
                        sq_v(f"v{hp}_{kc}", kc, ks, hp,
                             mk=f"V{hp}c{kc}" if ks == 3 else None)

            set_rel(0)
            sq_misc(lambda: load_w(wv, wv8))
            k_pass(0, "K0", skip0=True)       # chunks 1-3 of row 0
            if o.get("qa1_first", True):
                sq_load("qa1", xq8, 1)
                sq_kq(wq, "qa1", qT, 0, 1)
                sq_kq(wq, "qa1", qT, 1, 1, mk="QA1")
                sq_misc(load_wo)
                v_pass(0)
            else:
                v_pass(0)
                sq_misc(load_wo)
                sq_load("qa1", xq8, 1)
                sq_kq(wq, "qa1", qT, 0, 1)
                sq_kq(wq, "qa1", qT, 1, 1, mk="QA1")
            set_rel(1)
            sq_load("qa2", xq8, 2)
            sq_kq(wq, "qa2", qT, 0, 2)
            sq_kq(wq, "qa2", qT, 1, 2, mk="QA2")
            set_rel(2)
            sq_load("qa3", xq8, 3)
            sq_kq(wq, "qa3", qT, 0, 3)
            sq_kq(wq, "qa3", qT, 1, 3, mk="QA3")
            sq_load("qb0", xq8, 0)
            sq_kq(wq, "qb0", qT, 1, 0)
            k_pass(1, "K1")
            set_rel(4)
            v_pass(1)
            set_rel(5)
            sq_load("qc0", xq8, 0)
            sq_kq(wq, "qc0", qT, 2, 0)
            sq_kq(wq, "qc0", qT, 3, 0)
            k_pass(2, "K2")
            set_rel(7)
            v_pass(2)
            set_rel(8)
            sq_load("qd1", xq8, 1)
            sq_kq(wq, "qd1", qT, 2, 1)
            sq_kq(wq, "qd1", qT, 3, 1, mk="QB1")
            sq_load("qd2", xq8, 2)
            sq_kq(wq, "qd2", qT, 2, 2)
            sq_kq(wq, "qd2", qT, 3, 2, mk="QB2")
            set_rel(9)
            sq_load("qd3", xq8, 3)
            sq_kq(wq, "qd3", qT, 2, 3)
            sq_kq(wq, "qd3", qT, 3, 3, mk="QB3")
            set_rel(10)
            k_pass(3, "K3")
            set_rel(11)
            v_pass(3)

            # -- main loop (pr-major) -------------------------------------
            def qmarker(pr, qc):
                if qc == 0:
                    return None        # covered by startup / K-row forcing
                return f"QA{qc}" if pr <= 1 else f"QB{qc}"

            prev = None
            for pr in range(PT):
                for qc in range(QC):
                    cur_step[0] = 4 * pr + qc
                    if pr > 0 and qc == 0:
                        force(f"K{pr}")
                    mk = qmarker(pr, qc)
                    if mk:
                        force(mk)
                    if prev is not None:
                        ppr, pqc, pets = prev
                        lazy_v = o.get("lazy_v", True)
                        if pqc == 0 and not lazy_v:
                            force(f"V{ppr}c3")
                        ctx_kb = int(o.get("ctx_kb", 2))
                        for h in range(2):
                            for qs in range(QS):
                                push_ctx(ppr, pqc, pets, h, qs, ctx_kb,
                                         vrow=ppr
                                         if pqc == 0 and lazy_v else None)
                        if ppr == 3 and pqc < QC - 1:
                            for sc_ in range(pqc * 4, pqc * 4 + 4):
                                for jc in range(2):
                                    push_op_micro(sc_, jc, 2)
                    nstep = 4 * pr + qc + 1
                    npr, nqc = nstep // QC, nstep % QC
                    next_mks = []
                    if nstep < 16:
                        if nqc == 0 and npr > 0:
                            next_mks.append(f"K{npr}")
                        nq = qmarker(npr, nqc)
                        if nq:
                            next_mks.append(nq)
                        if nqc == 0:
                            next_mks.append(f"V{npr - 1}c3")
                    ets = phase_scores(pr, qc, dve_kbs=o["dve_kbs"],
                                       pool_kbs=o["pool_kbs"],
                                       next_mks=tuple(next_mks))
                    prev = (pr, qc, ets)
            # tail: last ctx phase + remaining outproj
            ppr, pqc, pets = prev
            for h in range(2):
                for qs in range(QS):
                    push_ctx(ppr, pqc, pets, h, qs, 0)
            tail_pool = o.get("tail_pool_copies", False)
            for i, sc_ in enumerate(range(12, 16)):
                for jc in range(2):
                    ceng = ("gpsimd" if tail_pool and (i + jc) % 2 else None)
                    push_op_micro(sc_, jc, 0, ceng=ceng)
            while work_q or script_q:
                drain(10**9)

    nc.compile()
    return nc


def _split8(x, scale):
    import ml_dtypes
    f8 = ml_dtypes.float8_e4m3
    xs = (x * scale).astype(np.float32)
    hi = xs.astype(f8)
    lo = (xs - hi.astype(np.float32)).astype(f8)
    return hi, lo


def _prep_inputs(query, key, value, Wq, Wk, Wv, Wo):
    """Per-core input maps; host does transposes, scaling, fp8 splits."""
    import ml_dtypes
    f8 = ml_dtypes.float8_e4m3

    def pack_x(x):
        # [S, D] -> xT [D, S] -> scaled hi/lo fp8 [D, 2, S]
        xT = np.ascontiguousarray(x.T)
        hi, lo = _split8(xT, SX)
        o = np.empty((D, 2, S), f8)
        o[:, 0], o[:, 1] = hi, lo
        return o

    def pack_w(Wrows):
        # Wrows [DHG, D]; device wants W^T [D, DHG] scaled hi/lo [D, 2, DHG]
        wT = np.ascontiguousarray(Wrows.T)
        hi, lo = _split8(wT, SW)
        o = np.empty((D, 2, DHG), f8)
        o[:, 0], o[:, 1] = hi, lo
        return o

    def pack_w_pt(Wrows):
        """Partition-major pt-sliced layout [2, PT, P, DT*128]: slice
        (i, pt)[p, dt*128+m] = W^T[dt*128+p, pt*128+m]."""
        DT, PT = D // P, DHG // P
        wT = np.ascontiguousarray(Wrows.T)
        hi, lo = _split8(wT, SW)
        o = np.empty((2, PT, P, DT * P), f8)
        for i, h in enumerate((hi, lo)):
            # [D, DHG] -> [DT, P, PT, 128] -> [PT, P, DT, 128]
            r = h.reshape(DT, P, PT, P).transpose(2, 1, 0, 3)
            o[i] = r.reshape(PT, P, DT * P)
        return o

    in_maps = []
    per_hg = {}
    for hg in range(2):
        lo_, hi_ = hg * DHG, (hg + 1) * DHG
        per_hg[hg] = {
            "wq8": pack_w_pt(Wq[lo_:hi_, :]),
            "wk8": pack_w_pt(Wk[lo_:hi_, :]),
            "wv8": pack_w(Wv[lo_:hi_, :]),
            "woT": np.ascontiguousarray(Wo[:, lo_:hi_].T).astype(np.float16),
        }
    per_b = {}
    for b in range(B):
        per_b[b] = {
            "xq8": pack_x(query[b]),
            "xk8": pack_x(key[b]),
            "xv8": pack_x(value[b]),
        }
    for c in range(N_CORES):
        b, hg = c // 2, c % 2
        in_maps.append({**per_b[b], **per_hg[hg]})
    return in_maps


def _reference_numpy(query, key, value, mask, Wq, Wk, Wv, Wo):
    """Correctness fallback for inputs the fast path doesn't handle."""
    out = np.empty((B, S, D), np.float32)
    for b in range(B):
        q = (query[b] @ Wq.T).reshape(S, H, HD).transpose(1, 0, 2)
        k = (key[b] @ Wk.T).reshape(S, H, HD).transpose(1, 0, 2)
        v = (value[b] @ Wv.T).reshape(S, H, HD).transpose(1, 0, 2)
        scores = np.einsum("hqd,hkd->hqk", q, k) / np.sqrt(np.float32(HD))
        scores = np.where(mask[b][None, :, :] == 0, -np.inf, scores)
        scores = scores - scores.max(axis=-1, keepdims=True)
        e = np.exp(scores)
        attn = e / e.sum(axis=-1, keepdims=True)
        ctx = np.einsum("hqk,hkd->hqd", attn, v)
        out[b] = ctx.transpose(1, 0, 2).reshape(S, D) @ Wo.T
    return out


def run_device(query, key, value, Wq, Wk, Wv, Wo, trace=False,
               trace_kwargs=None, debug=False, opts=None):
    from concourse.bass_utils import run_bass_kernel_spmd

    key_ = ("nc", debug)
    if key_ not in _CACHE:
        _CACHE[key_] = _build_nc(debug, opts=opts)
    nc = _CACHE[key_]
    in_maps = _prep_inputs(query, key, value, Wq, Wk, Wv, Wo)
    res = run_bass_kernel_spmd(nc, in_maps, list(range(N_CORES)),
                               trace=trace, **(trace_kwargs or {}))
    out = np.empty((B, S, D), np.float32)
    for b in range(B):
        out[b] = res.results[2 * b]["out"] + res.results[2 * b + 1]["out"]
    return out, res


def kernel(query, key, value, mask, Wq, Wk, Wv, Wo):
    query = np.asarray(query, np.float32)
    key = np.asarray(key, np.float32)
    value = np.asarray(value, np.float32)
    Wq = np.asarray(Wq, np.float32)
    Wk = np.asarray(Wk, np.float32)
    Wv = np.asarray(Wv, np.float32)
    Wo = np.asarray(Wo, np.float32)
    if not np.all(np.asarray(mask) == 1):
        return _reference_numpy(query, key, value, np.asarray(mask),
                                Wq, Wk, Wv, Wo)
    out, _ = run_device(query, key, value, Wq, Wk, Wv, Wo)
    return out

